# revision 8
# baseline (speedup 1.0000x reference)
"""BC-LSTM Trainium2 kernel: data-parallel over batch on 8 NeuronCores.

Shapes (hardcoded): B=256, T=128, IN_DIMS=[300,100,512], HID=[128,64,128],
FC=[100,50,100], DH=256, DF=128, NC=6. Per-core batch shard b=32.

v2 layout strategy (per core):
- Host pre-transposes activations/weights; g-gate columns pre-scaled x2 so
  tanh(x) = 2*sigmoid(2x)-1 turns every scan activation into ONE sigmoid
  table-set (no ACT table thrash, fewer ACT ops).
- Cell state carried as C = 2c so tanh(c) = 2*sigmoid(C)-1 stays one sigmoid.
- Input projections Z computed in bulk [128(4t x 32b), 4H] chunks on PE,
  evacuated PSUM->SBUF split across DVE/ACT.
- Modality scans packed on partitions 0..95; gates [96,512] from identity
  Z-gather + h-stationary matmuls at distinct PE col bands (concurrent).
- h fed back via PE transpose (no DMA transpose); the h-tanh is fused into
  the PSUM evacuation as sigmoid + (2x-1) tensor_scalar.
- Dialogue gates packed [64,512] one PSUM bank: rows 0-31 = (i|f), rows
  32-63 = (o|g) -> ONE sigmoid per step.
- log_softmax deferred to one end-pass (Exp/Ln tables loaded once).
"""

import sys

sys.path.insert(0, "/opt/trn_rl_repo")

import numpy as np
import ml_dtypes

import concourse.bass as bass
import concourse.tile as tile
from concourse import bacc, mybir
from concourse.bass_utils import run_bass_kernel_spmd

F32 = mybir.dt.float32
BF16 = mybir.dt.bfloat16
AF = mybir.ActivationFunctionType
ALU = mybir.AluOpType

NCORES = 8
B, T = 256, 128
BSH = B // NCORES  # 32
TB = T * BSH  # 4096
IN_DIMS = [300, 100, 512]
HID = [128, 64, 128]
FCD = [100, 50, 100]
DH, DF, NCLS = 256, 128, 6
GP = 128  # per-gate padded width for modality scans
NCH = 32  # chunks
TC = 4  # timesteps per chunk (TC*BSH = 128 rows)

DPAD = [384, 128, 512]  # mod0: 300+bias_row+pad, mod1: 100+bias_row+pad, mod2: exact
KCH = [3, 1, 4]  # number of 128-row K chunks
GW = [128, 64, 128]  # per-gate column width of the packed Z/gate matmuls


def _gate_reorder_T(w, H, P):
    """w [4H, D] torch gate order (i,f,g,o) -> W.T [D, 4P] order (i,f,o,g),
    each gate padded to P columns. The g gate is scaled x2 (tanh-via-sigmoid)."""
    D = w.shape[1]
    out = np.zeros((D, 4 * P), np.float32)
    for gi, src in enumerate([0, 1, 3, 2]):
        out[:, gi * P : gi * P + H] = w[src * H : (src + 1) * H, :].T
    out[:, 3 * P : 4 * P] *= 2.0
    return out


def _gate_reorder_b_w(bvec, H, P):
    out = np.zeros(4 * P, np.float32)
    for gi, src in enumerate([0, 1, 3, 2]):
        out[gi * P : gi * P + H] = bvec[src * H : (src + 1) * H]
    out[3 * P : 4 * P] *= 2.0
    return out


def _gate_reorder_b(bvec, H, P):
    out = np.zeros(4 * P, np.float32)
    for gi, src in enumerate([0, 1, 3, 2]):
        out[gi * P : gi * P + H] = bvec[src * H : (src + 1) * H]
    out[3 * P : 4 * P] *= 2.0
    return out


def _bf16(x):
    return np.ascontiguousarray(x).astype(ml_dtypes.bfloat16)


_CACHE = {}


def _build():
    if "nc" in _CACHE:
        return _CACHE["nc"]
    nc = bacc.Bacc("TRN2", target_bir_lowering=False, debug=False, num_devices=NCORES)

    def din(name, shape, dt=BF16):
        return nc.dram_tensor(name, shape, dt, kind="ExternalInput").ap()

    # per-core inputs
    xt = [din(f"xt{s}", [DPAD[s], TB]) for s in range(3)]
    wih = [din(f"wih{s}", [DPAD[s], 4 * GW[s]]) for s in range(3)]
    bias2t = din("bias2t", [128, 4 * GP], F32)
    whh = [din(f"whh{s}", [HID[s], 4 * GW[s]]) for s in range(3)]
    fcw = [din(f"fcw{s}", [HID[s], FCD[s]]) for s in range(3)]
    fcb = [din(f"fcb{s}", [FCD[s], 1], F32) for s in range(3)]
    wihd = [din(f"wihd{s}", [FCD[s], 4 * DH]) for s in range(3)]
    bdrow2 = din("bdrow2", [128, 4 * DH], F32)
    whhd = [din(f"whhd{k}", [128, 4 * DH]) for k in range(2)]
    fcoutw = [din(f"fcoutw{k}", [128, DF]) for k in range(2)]
    fcoutb = din("fcoutb", [DF, 1], F32)
    smaxwt = din("smaxwt", [DF, NCLS])
    smaxbt = din("smaxbt", [128, NCLS], F32)
    idbf = din("idbf", [128, 128])
    i32s = din("i32s", [128, 32])
    out = nc.dram_tensor("out", [BSH, T, NCLS], F32, kind="ExternalOutput").ap()

    with tile.TileContext(nc) as tc, bass.ExitStack() as ctx:
        ep = ctx.enter_context
        stat = ep(tc.tile_pool(name="stat", bufs=1))
        sb = {}
        # small weights first (so the big xt loads don't delay them)
        for s in range(3):
            sb[f"wih{s}"] = stat.tile([128, KCH[s] * 4 * GW[s]], BF16, tag=f"wih{s}", name=f"wih{s}")
            for k in range(KCH[s]):
                nc.sync.dma_start(
                    sb[f"wih{s}"][:, k * 4 * GW[s] : (k + 1) * 4 * GW[s]],
                    wih[s][k * 128 : (k + 1) * 128, :],
                )
            sb[f"whh{s}"] = stat.tile([HID[s], 4 * GW[s]], BF16, tag=f"whh{s}", name=f"whh{s}")
            nc.sync.dma_start(sb[f"whh{s}"][:], whh[s][:])
            sb[f"fcw{s}"] = stat.tile([HID[s], FCD[s]], BF16, tag=f"fcw{s}", name=f"fcw{s}")
            nc.sync.dma_start(sb[f"fcw{s}"][:], fcw[s][:])
            sb[f"fcb{s}"] = stat.tile([FCD[s], 1], F32, tag=f"fcb{s}", name=f"fcb{s}")
            nc.sync.dma_start(sb[f"fcb{s}"][:], fcb[s][:])
            sb[f"wihd{s}"] = stat.tile([FCD[s], 4 * DH], BF16, tag=f"wihd{s}", name=f"wihd{s}")
            nc.sync.dma_start(sb[f"wihd{s}"][:], wihd[s][:])
        for name, src, shp, dt in [
            ("bias2t", bias2t, [128, 4 * GP], F32),
            ("bdrow2", bdrow2, [128, 4 * DH], F32),
            ("fcoutb", fcoutb, [DF, 1], F32),
            ("smaxwt", smaxwt, [DF, NCLS], BF16),
            ("smaxbt", smaxbt, [128, NCLS], F32),
            ("idbf", idbf, [128, 128], BF16),
            ("i32s", i32s, [128, 32], BF16),
        ]:
            sb[name] = stat.tile(shp, dt, tag=name, name=name)
            nc.sync.dma_start(sb[name][:], src[:])
        for k in range(2):
            sb[f"whhd{k}"] = stat.tile([128, 4 * DH], BF16, tag=f"whhd{k}", name=f"whhd{k}")
            nc.sync.dma_start(sb[f"whhd{k}"][:], whhd[k][:])
            sb[f"fcoutw{k}"] = stat.tile([128, DF], BF16, tag=f"fcoutw{k}", name=f"fcoutw{k}")
            nc.sync.dma_start(sb[f"fcoutw{k}"][:], fcoutw[k][:])
        # big activation loads, split by column quarter so chunk 0 starts early
        for s in range(3):
            sb[f"xt{s}"] = stat.tile([128, KCH[s] * TB], BF16, tag=f"xt{s}", name=f"xt{s}")
        QW = TB // 4
        for q in range(4):
            for s in range(3):
                for k in range(KCH[s]):
                    nc.sync.dma_start(
                        sb[f"xt{s}"][:, k * TB + q * QW : k * TB + (q + 1) * QW],
                        xt[s][k * 128 : (k + 1) * 128, q * QW : (q + 1) * QW],
                    )

        # history buffers (block t holds state BEFORE step t; block t+1 = output of step t)
        hmt = stat.tile([128, (T + 1) * 96], BF16, tag="hmt")
        hdt = stat.tile([128, (T + 1) * 64], BF16, tag="hdt")
        c3 = stat.tile([96, GP], BF16, tag="c3")  # C = 2c for the 3 mod scans
        cd = stat.tile([32, DH], BF16, tag="cd")  # C = 2c for the dialogue scan
        lgt = stat.tile([128, NCH * NCLS], F32, tag="lgt")  # logits staging
        nc.vector.memset(hmt[:, 0:96], 0.0)
        nc.vector.memset(hdt[:, 0:64], 0.0)
        nc.vector.memset(c3[:], 0.0)
        nc.vector.memset(cd[:], 0.0)

        zsb = ep(tc.tile_pool(name="zsb", bufs=2))
        zdpool = ep(tc.tile_pool(name="zdpool", bufs=2))
        fpool = ep(tc.tile_pool(name="fpool", bufs=2))
        ew = ep(tc.tile_pool(name="ew", bufs=2))
        smp = ep(tc.tile_pool(name="smp", bufs=1))
        ps_m = ep(tc.tile_pool(name="ps_m", bufs=2, space="PSUM"))
        ps_d = ep(tc.tile_pool(name="ps_d", bufs=1, space="PSUM"))
        ps1 = ep(tc.tile_pool(name="ps1", bufs=3, space="PSUM"))
        ps_t = ep(tc.tile_pool(name="ps_t", bufs=1, space="PSUM"))

        def inproj_scan(c, s):
            zp = ps1.tile([128, 4 * GW[s]], F32, tag="ps", name="zp")
            for k in range(KCH[s]):
                nc.tensor.matmul(
                    zp[:],
                    sb[f"xt{s}"][:, k * TB + c * 128 : k * TB + (c + 1) * 128],
                    sb[f"wih{s}"][:, k * 4 * GW[s] : (k + 1) * 4 * GW[s]],
                    start=(k == 0),
                    stop=(k == KCH[s] - 1),
                )
            z = zsb.tile([128, 4 * GW[s]], BF16, tag=f"z{s}", name="z")
            if s == 2:
                nc.vector.tensor_add(z[:], zp[:], sb["bias2t"][:])
            elif s == 1:
                nc.scalar.copy(z[:], zp[:])
            else:
                nc.vector.tensor_copy(z[:], zp[:])
            return z

        def mod_step(t, zt):
            trel = t % TC
            gp = ps_m.tile([96, 4 * GP], F32, tag="gm", name="gp")
            for s in range(3):
                if GW[s] == GP:
                    gout = gp[32 * s : 32 * s + 32, :]
                else:
                    gout = gp[32 * s : 32 * s + 32, :].rearrange(
                        "p (g x) -> p g x", x=GP
                    )[:, :, 0 : GW[s]]
                nc.tensor.matmul(
                    gout,
                    sb["i32s"][32 * trel : 32 * trel + 32, :],
                    zt[s][32 * trel : 32 * trel + 32, :],
                    start=True,
                    stop=False,
                    tile_position=(32 * trel, 32 * s),
                )
                nc.tensor.matmul(
                    gout,
                    hmt[0 : HID[s], t * 96 + 32 * s : t * 96 + 32 * s + 32],
                    sb[f"whh{s}"][0 : HID[s], :],
                    start=False,
                    stop=True,
                    tile_position=(0, 32 * s),
                )
            # gates: (i | f | o | 2g); one sigmoid covers all four
            sg = ew.tile([96, 4 * GP], BF16, tag="sg", name="sg")
            nc.scalar.activation(sg[:], gp[:], AF.Sigmoid)
            g2 = ew.tile([96, GP], BF16, tag="g2", name="g2")  # 2*tanh(g)
            nc.vector.tensor_scalar(g2[:], sg[:, 3 * GP : 4 * GP], 4.0, -2.0, ALU.mult, ALU.add)
            m1 = ew.tile([96, GP], BF16, tag="m1", name="m1")
            nc.vector.tensor_mul(m1[:], sg[:, GP : 2 * GP], c3[:])  # f * C
            m2 = ew.tile([96, GP], BF16, tag="m2", name="m2")
            nc.gpsimd.tensor_mul(m2[:], sg[:, 0:GP], g2[:])  # i * 2tanh(g)
            nc.vector.tensor_add(c3[:], m1[:], m2[:])  # C' = 2c'
            sc = ew.tile([96, GP], BF16, tag="sc", name="sc")
            nc.scalar.activation(sc[:], c3[:], AF.Sigmoid)
            tc2 = ew.tile([96, GP], BF16, tag="tc2", name="tc2")  # 2*tanh(c)
            nc.vector.tensor_scalar(tc2[:], sc[:], 4.0, -2.0, ALU.mult, ALU.add)
            h2d = ew.tile([96, GP], BF16, tag="h2d", name="h2d")  # 2*h2
            nc.vector.tensor_mul(h2d[:], sg[:, 2 * GP : 3 * GP], tc2[:])
            pt = ps_t.tile([128, 160], BF16, tag="tr", name="pt")
            nc.tensor.transpose(pt[:, 0:96], h2d[:], sb["idbf"][0:96, 0:96])
            se = ew.tile([128, 96], BF16, tag="se", name="se")
            nc.scalar.activation(se[:], pt[:, 0:96], AF.Sigmoid)  # sigma(2*h2)
            nc.vector.tensor_scalar(
                hmt[:, (t + 1) * 96 : (t + 2) * 96], se[:], 2.0, -1.0, ALU.mult, ALU.add
            )  # tanh(h2)

        def fc_piece(c, s):
            fp = ps1.tile([FCD[s], 128], F32, tag="ps", name="fp")
            nc.tensor.matmul(
                fp[:],
                sb[f"fcw{s}"][:],
                hmt_b[0 : HID[s], c * TC + 1 : c * TC + 5, 32 * s : 32 * s + 32],
                start=True,
                stop=True,
            )
            ft = fpool.tile([FCD[s], 128], BF16, tag=f"ft{s}", name="ft")
            nc.scalar.activation(ft[:], fp[:], AF.Tanh, bias=sb[f"fcb{s}"][:])
            return ft

        def zd_half(fts, zd, h):
            zdp = ps1.tile([128, 512], F32, tag="ps", name="zdp")
            sl = slice(512 * h, 512 * (h + 1))
            for s in range(3):
                nc.tensor.matmul(
                    zdp[:], fts[s][:], sb[f"wihd{s}"][:, sl],
                    start=(s == 0), stop=(s == 2),
                )
            nc.vector.tensor_add(zd[:, sl], zdp[:], sb["bdrow2"][:, sl])

        def dial_inject(zd):
            gd = ps_d.tile([128, 4 * DH], F32, tag="gd", name="gd")
            for hh in range(2):
                sl = slice(512 * hh, 512 * (hh + 1))
                nc.tensor.matmul(
                    gd[:, sl], sb["idbf"][:], zd[:, sl],
                    start=True, stop=False, skip_group_check=True,
                )
            return gd

        def dial_step(t, gd):
            trel = t % TC
            for hh in range(2):
                sl = slice(512 * hh, 512 * (hh + 1))
                for k in range(2):
                    nc.tensor.matmul(
                        gd[32 * trel : 32 * trel + 32, sl],
                        hdt[:, t * 64 + 32 * k : t * 64 + 32 * k + 32],
                        sb[f"whhd{k}"][:, sl],
                        start=False,
                        stop=(trel == TC - 1 and k == 1),
                        tile_position=(0, 32 * trel),
                        skip_group_check=True,
                    )
            sgd = ew.tile([32, 4 * DH], BF16, tag="sgd", name="sgd")
            nc.scalar.activation(sgd[:], gd[32 * trel : 32 * trel + 32, :], AF.Sigmoid)
            g2d = ew.tile([32, DH], BF16, tag="g2d", name="g2d")  # 2*tanh(g)
            nc.vector.tensor_scalar(g2d[:], sgd[:, 3 * DH : 4 * DH], 4.0, -2.0, ALU.mult, ALU.add)
            m2 = ew.tile([32, DH], BF16, tag="m2d", name="m2d")
            nc.vector.tensor_mul(m2[:], sgd[:, 0:DH], g2d[:])  # i * 2tanh(g)
            m1 = ew.tile([32, DH], BF16, tag="m1d", name="m1d")
            nc.gpsimd.tensor_mul(m1[:], sgd[:, DH : 2 * DH], cd[:])  # f * C
            nc.vector.tensor_add(cd[:], m1[:], m2[:])
            scd = ew.tile([32, DH], BF16, tag="scd", name="scd")
            nc.scalar.activation(scd[:], cd[:], AF.Sigmoid)
            tcd = ew.tile([32, DH], BF16, tag="tcd", name="tcd")
            nc.vector.tensor_scalar(tcd[:], scd[:], 2.0, -1.0, ALU.mult, ALU.add)
            h2 = ew.tile([32, DH], BF16, tag="h2", name="h2")
            nc.vector.tensor_mul(h2[:], sgd[:, 2 * DH : 3 * DH], tcd[:])  # o * tanh(c)
            ptd = ps_t.tile([128, 160], BF16, tag="tr", name="ptd")
            for k in range(2):
                nc.tensor.transpose(
                    ptd[:, 32 * k : 32 * (k + 1)],
                    h2[:, 128 * k : 128 * (k + 1)],
                    sb["idbf"][0:32, 0:32],
                )
            nc.vector.tensor_copy(hdt[:, (t + 1) * 64 : (t + 2) * 64], ptd[:, 0:64])

        def head(c):
            hp = ps1.tile([DF, 128], F32, tag="ps", name="hp")
            for k in range(2):
                nc.tensor.matmul(
                    hp[:],
                    sb[f"fcoutw{k}"][:],
                    hdt_b[:, c * TC + 1 : c * TC + 5, 32 * k : 32 * k + 32],
                    start=(k == 0),
                    stop=(k == 1),
                )
            hst = fpool.tile([DF, 128], BF16, tag="hst", name="hst")
            nc.scalar.activation(hst[:], hp[:], AF.Tanh, bias=sb["fcoutb"][:])
            lp = ps1.tile([128, NCLS], F32, tag="ps", name="lp")
            nc.tensor.matmul(lp[:], hst[:], sb["smaxwt"][:], start=True, stop=True)
            nc.vector.tensor_add(lgt[:, NCLS * c : NCLS * (c + 1)], lp[:], sb["smaxbt"][:])

        def endpass():
            lg3 = lgt[:].rearrange("p (c j) -> p c j", j=NCLS)
            mx = smp.tile([128, NCH], F32, tag="mx", name="mx")
            nc.vector.tensor_reduce(mx[:].unsqueeze(2), lg3, mybir.AxisListType.X, ALU.max)
            mxb = mx[:].unsqueeze(2).broadcast_to([128, NCH, NCLS])
            lc = smp.tile([128, NCH * NCLS], F32, tag="lc", name="lc")
            lc3 = lc[:].rearrange("p (c j) -> p c j", j=NCLS)
            nc.vector.tensor_sub(lc3, lg3, mxb)
            ex = smp.tile([128, NCH * NCLS], F32, tag="ex", name="ex")
            nc.scalar.activation(ex[:], lc[:], AF.Exp)
            se = smp.tile([128, NCH], F32, tag="sme", name="sme")
            nc.vector.tensor_reduce(
                se[:].unsqueeze(2), ex[:].rearrange("p (c j) -> p c j", j=NCLS),
                mybir.AxisListType.X, ALU.add,
            )
            lns = smp.tile([128, NCH], F32, tag="lns", name="lns")
            nc.scalar.activation(lns[:], se[:], AF.Ln)
            fin = smp.tile([128, NCH * NCLS], F32, tag="fin", name="fin")
            nc.vector.tensor_sub(
                fin[:].rearrange("p (c j) -> p c j", j=NCLS),
                lc3,
                lns[:].unsqueeze(2).broadcast_to([128, NCH, NCLS]),
            )
            for c in range(NCH):
                nc.sync.dma_start(
                    out[:, c * TC : (c + 1) * TC, :].rearrange("i t c -> t i c"),
                    fin[:, NCLS * c : NCLS * (c + 1)],
                )

        hmt_b = hmt[:].rearrange("p (t g) -> p t g", g=96)
        hdt_b = hdt[:].rearrange("p (t g) -> p t g", g=64)

        # software-pipelined: inproj runs 1 chunk ahead, dialogue lags 2 chunks,
        # bulk matmul groups woven between scan steps to keep PE streaming.
        ztcur = [inproj_scan(0, s) for s in range(3)]
        ztnext = [None] * 3
        fts = [None] * 3
        gdcur = gdnext = zdw = None
        for c in range(NCH + 3):
            for trel in range(TC):
                if c < NCH:
                    mod_step(c * TC + trel, ztcur)
                if c + 1 < NCH and trel < 3:
                    ztnext[trel] = inproj_scan(c + 1, trel)
                if 2 <= c < NCH + 2:
                    dial_step((c - 2) * TC + trel, gdcur)
                if 1 <= c <= NCH:
                    if trel == 0:
                        fts[0] = fc_piece(c - 1, 0)
                        fts[1] = fc_piece(c - 1, 1)
                    elif trel == 1:
                        fts[2] = fc_piece(c - 1, 2)
                        zdw = zdpool.tile([128, 4 * DH], BF16, tag="zd", name="zd")
                        zd_half(fts, zdw, 0)
                    elif trel == 2:
                        zd_half(fts, zdw, 1)
                if 3 <= c < NCH + 3 and trel == 3:
                    head(c - 3)
            ztcur, ztnext = ztnext, [None] * 3
            if 1 <= c <= NCH:
                gdnext = dial_inject(zdw)
            gdcur = gdnext
        endpass()

    nc.compile()
    _CACHE["nc"] = nc
    return nc


def _prep_core(inputs, core):
    """Build the per-core input map (host-side shard/transpose/pad/bf16)."""
    d = {}
    sl = slice(core * BSH, (core + 1) * BSH)
    for s in range(3):
        D = IN_DIMS[s]
        shard = np.asarray(inputs[f"mod{s}"][sl], np.float32)  # [32, T, D]
        xts = np.zeros((DPAD[s], TB), np.float32)
        xts[:D] = shard.transpose(2, 1, 0).reshape(D, TB)
        gw = GW[s]
        wt = np.zeros((DPAD[s], 4 * gw), np.float32)
        wt[:D] = _gate_reorder_T(np.asarray(inputs[f"w_ih{s}"], np.float32), HID[s], gw)
        bias = _gate_reorder_b_w(
            np.asarray(inputs[f"b_ih{s}"], np.float32)
            + np.asarray(inputs[f"b_hh{s}"], np.float32),
            HID[s],
            gw,
        )
        if s == 2:
            d["bias2t"] = np.broadcast_to(bias, (128, 4 * gw)).copy()
        else:
            xts[D] = 1.0
            wt[D] = bias
        d[f"xt{s}"] = _bf16(xts)
        d[f"wih{s}"] = _bf16(wt)
        d[f"whh{s}"] = _bf16(
            _gate_reorder_T(np.asarray(inputs[f"w_hh{s}"], np.float32), HID[s], gw)
        )
        d[f"fcw{s}"] = _bf16(np.asarray(inputs[f"fc_w{s}"], np.float32).T)
        d[f"fcb{s}"] = np.asarray(inputs[f"fc_b{s}"], np.float32).reshape(-1, 1).copy()
    wihdt = _gate_reorder_T(np.asarray(inputs["w_ih_d"], np.float32), DH, DH)  # [250, 1024]
    d["wihd0"] = _bf16(wihdt[0:100])
    d["wihd1"] = _bf16(wihdt[100:150])
    d["wihd2"] = _bf16(wihdt[150:250])
    bd = _gate_reorder_b(
        np.asarray(inputs["b_ih_d"], np.float32)
        + np.asarray(inputs["b_hh_d"], np.float32),
        DH,
        DH,
    )
    d["bdrow2"] = np.broadcast_to(bd, (128, 4 * DH)).copy()
    whhdt = _gate_reorder_T(np.asarray(inputs["w_hh_d"], np.float32), DH, DH)  # [256, 1024]
    d["whhd0"] = _bf16(whhdt[0:128])
    d["whhd1"] = _bf16(whhdt[128:256])
    fow = np.asarray(inputs["fc_out_w"], np.float32).T  # [256, 128]
    d["fcoutw0"] = _bf16(fow[0:128])
    d["fcoutw1"] = _bf16(fow[128:256])
    d["fcoutb"] = np.asarray(inputs["fc_out_b"], np.float32).reshape(-1, 1).copy()
    d["smaxwt"] = _bf16(np.asarray(inputs["smax_w"], np.float32).T)
    d["smaxbt"] = np.broadcast_to(
        np.asarray(inputs["smax_b"], np.float32), (128, NCLS)
    ).copy()
    d["idbf"] = _bf16(np.eye(128, dtype=np.float32))
    i32 = np.zeros((128, 32), np.float32)
    for k in range(4):
        i32[32 * k : 32 * (k + 1)] = np.eye(32)
    d["i32s"] = _bf16(i32)
    return d


def run(inputs, trace=False, **kw):
    nc = _build()
    in_maps = [_prep_core(inputs, i) for i in range(NCORES)]
    res = run_bass_kernel_spmd(nc, in_maps, list(range(NCORES)), trace=trace, **kw)
    full = np.concatenate(
        [np.asarray(res.results[i]["out"], np.float32) for i in range(NCORES)], axis=0
    )
    return full, res


def kernel(**inputs) -> np.ndarray:
    out, _ = run(inputs, trace=False)
    return out


# revision 9
# speedup vs baseline: 1.0172x; 1.0172x over previous
"""BC-LSTM Trainium2 kernel: data-parallel over batch on 8 NeuronCores.

Shapes (hardcoded): B=256, T=128, IN_DIMS=[300,100,512], HID=[128,64,128],
FC=[100,50,100], DH=256, DF=128, NC=6. Per-core batch shard b=32.

v2 layout strategy (per core):
- Host pre-transposes activations/weights; g-gate columns pre-scaled x2 so
  tanh(x) = 2*sigmoid(2x)-1 turns every scan activation into ONE sigmoid
  table-set (no ACT table thrash, fewer ACT ops).
- Cell state carried as C = 2c so tanh(c) = 2*sigmoid(C)-1 stays one sigmoid.
- Input projections Z computed in bulk [128(4t x 32b), 4H] chunks on PE,
  evacuated PSUM->SBUF split across DVE/ACT.
- Modality scans packed on partitions 0..95; gates [96,512] from identity
  Z-gather + h-stationary matmuls at distinct PE col bands (concurrent).
- h fed back via PE transpose (no DMA transpose); the h-tanh is fused into
  the PSUM evacuation as sigmoid + (2x-1) tensor_scalar.
- Dialogue gates packed [64,512] one PSUM bank: rows 0-31 = (i|f), rows
  32-63 = (o|g) -> ONE sigmoid per step.
- log_softmax deferred to one end-pass (Exp/Ln tables loaded once).
"""

import sys

sys.path.insert(0, "/opt/trn_rl_repo")

import numpy as np
import ml_dtypes

import concourse.bass as bass
import concourse.tile as tile
from concourse import bacc, mybir
from concourse.bass_utils import run_bass_kernel_spmd

F32 = mybir.dt.float32
BF16 = mybir.dt.bfloat16
AF = mybir.ActivationFunctionType
ALU = mybir.AluOpType

NCORES = 8
B, T = 256, 128
BSH = B // NCORES  # 32
TB = T * BSH  # 4096
IN_DIMS = [300, 100, 512]
HID = [128, 64, 128]
FCD = [100, 50, 100]
DH, DF, NCLS = 256, 128, 6
GP = 128  # per-gate padded width for modality scans
NCH = 32  # chunks
TC = 4  # timesteps per chunk (TC*BSH = 128 rows)

DPAD = [384, 128, 512]  # mod0: 300+bias_row+pad, mod1: 100+bias_row+pad, mod2: exact
KCH = [3, 1, 4]  # number of 128-row K chunks
GW = [128, 64, 128]  # per-gate column width of the packed Z/gate matmuls


def _gate_reorder_T(w, H, P):
    """w [4H, D] torch gate order (i,f,g,o) -> W.T [D, 4P] order (i,f,o,g),
    each gate padded to P columns. The g gate is scaled x2 (tanh-via-sigmoid)."""
    D = w.shape[1]
    out = np.zeros((D, 4 * P), np.float32)
    for gi, src in enumerate([0, 1, 3, 2]):
        out[:, gi * P : gi * P + H] = w[src * H : (src + 1) * H, :].T
    out[:, 3 * P : 4 * P] *= 2.0
    return out


def _gate_reorder_b_w(bvec, H, P):
    out = np.zeros(4 * P, np.float32)
    for gi, src in enumerate([0, 1, 3, 2]):
        out[gi * P : gi * P + H] = bvec[src * H : (src + 1) * H]
    out[3 * P : 4 * P] *= 2.0
    return out


def _gate_reorder_b(bvec, H, P):
    out = np.zeros(4 * P, np.float32)
    for gi, src in enumerate([0, 1, 3, 2]):
        out[gi * P : gi * P + H] = bvec[src * H : (src + 1) * H]
    out[3 * P : 4 * P] *= 2.0
    return out


def _bf16(x):
    return np.ascontiguousarray(x).astype(ml_dtypes.bfloat16)


_CACHE = {}


def _build():
    if "nc" in _CACHE:
        return _CACHE["nc"]
    nc = bacc.Bacc("TRN2", target_bir_lowering=False, debug=False, num_devices=NCORES)

    def din(name, shape, dt=BF16):
        return nc.dram_tensor(name, shape, dt, kind="ExternalInput").ap()

    # per-core inputs
    xt = [din(f"xt{s}", [DPAD[s], TB]) for s in range(3)]
    wih = [din(f"wih{s}", [DPAD[s], 4 * GW[s]]) for s in range(3)]
    bias2t = din("bias2t", [128, 4 * GP], F32)
    whh = [din(f"whh{s}", [HID[s], 4 * GW[s]]) for s in range(3)]
    fcw = [din(f"fcw{s}", [HID[s], FCD[s]]) for s in range(3)]
    fcb = [din(f"fcb{s}", [FCD[s], 1], F32) for s in range(3)]
    wihd = [din(f"wihd{s}", [FCD[s], 4 * DH]) for s in range(3)]
    bdrow2 = din("bdrow2", [128, 4 * DH], F32)
    whhd = [din(f"whhd{k}", [128, 4 * DH]) for k in range(2)]
    fcoutw = [din(f"fcoutw{k}", [128, DF]) for k in range(2)]
    fcoutb = din("fcoutb", [DF, 1], F32)
    smaxwt = din("smaxwt", [DF, NCLS])
    smaxbt = din("smaxbt", [128, NCLS], F32)
    idbf = din("idbf", [128, 128])
    i32s = din("i32s", [128, 32])
    out = nc.dram_tensor("out", [BSH, T, NCLS], F32, kind="ExternalOutput").ap()

    with tile.TileContext(nc) as tc, bass.ExitStack() as ctx:
        ep = ctx.enter_context
        stat = ep(tc.tile_pool(name="stat", bufs=1))
        sb = {}
        # small weights first (so the big xt loads don't delay them)
        for s in range(3):
            sb[f"wih{s}"] = stat.tile([128, KCH[s] * 4 * GW[s]], BF16, tag=f"wih{s}", name=f"wih{s}")
            for k in range(KCH[s]):
                nc.sync.dma_start(
                    sb[f"wih{s}"][:, k * 4 * GW[s] : (k + 1) * 4 * GW[s]],
                    wih[s][k * 128 : (k + 1) * 128, :],
                )
            sb[f"whh{s}"] = stat.tile([HID[s], 4 * GW[s]], BF16, tag=f"whh{s}", name=f"whh{s}")
            nc.sync.dma_start(sb[f"whh{s}"][:], whh[s][:])
            sb[f"fcw{s}"] = stat.tile([HID[s], FCD[s]], BF16, tag=f"fcw{s}", name=f"fcw{s}")
            nc.sync.dma_start(sb[f"fcw{s}"][:], fcw[s][:])
            sb[f"fcb{s}"] = stat.tile([FCD[s], 1], F32, tag=f"fcb{s}", name=f"fcb{s}")
            nc.sync.dma_start(sb[f"fcb{s}"][:], fcb[s][:])
            sb[f"wihd{s}"] = stat.tile([FCD[s], 4 * DH], BF16, tag=f"wihd{s}", name=f"wihd{s}")
            nc.sync.dma_start(sb[f"wihd{s}"][:], wihd[s][:])
        for name, src, shp, dt in [
            ("bias2t", bias2t, [128, 4 * GP], F32),
            ("bdrow2", bdrow2, [128, 4 * DH], F32),
            ("fcoutb", fcoutb, [DF, 1], F32),
            ("smaxwt", smaxwt, [DF, NCLS], BF16),
            ("smaxbt", smaxbt, [128, NCLS], F32),
            ("idbf", idbf, [128, 128], BF16),
            ("i32s", i32s, [128, 32], BF16),
        ]:
            sb[name] = stat.tile(shp, dt, tag=name, name=name)
            nc.sync.dma_start(sb[name][:], src[:])
        for k in range(2):
            sb[f"whhd{k}"] = stat.tile([128, 4 * DH], BF16, tag=f"whhd{k}", name=f"whhd{k}")
            nc.sync.dma_start(sb[f"whhd{k}"][:], whhd[k][:])
            sb[f"fcoutw{k}"] = stat.tile([128, DF], BF16, tag=f"fcoutw{k}", name=f"fcoutw{k}")
            nc.sync.dma_start(sb[f"fcoutw{k}"][:], fcoutw[k][:])
        # big activation loads, split by column quarter so chunk 0 starts early
        for s in range(3):
            sb[f"xt{s}"] = stat.tile([128, KCH[s] * TB], BF16, tag=f"xt{s}", name=f"xt{s}")
        QW = TB // 4
        for q in range(4):
            for s in range(3):
                for k in range(KCH[s]):
                    nc.sync.dma_start(
                        sb[f"xt{s}"][:, k * TB + q * QW : k * TB + (q + 1) * QW],
                        xt[s][k * 128 : (k + 1) * 128, q * QW : (q + 1) * QW],
                    )

        # history buffers (block t holds state BEFORE step t; block t+1 = output of step t)
        hmt = stat.tile([128, (T + 1) * 96], BF16, tag="hmt")
        hdt = stat.tile([128, (T + 1) * 64], BF16, tag="hdt")
        c3 = stat.tile([96, GP], BF16, tag="c3")  # C = 2c for the 3 mod scans
        cd = stat.tile([32, DH], BF16, tag="cd")  # C = 2c for the dialogue scan
        lgt = stat.tile([128, NCH * NCLS], F32, tag="lgt")  # logits staging
        nc.vector.memset(hmt[:, 0:96], 0.0)
        nc.vector.memset(hdt[:, 0:64], 0.0)
        nc.vector.memset(c3[:], 0.0)
        nc.vector.memset(cd[:], 0.0)

        zsb = ep(tc.tile_pool(name="zsb", bufs=2))
        zdpool = ep(tc.tile_pool(name="zdpool", bufs=2))
        fpool = ep(tc.tile_pool(name="fpool", bufs=2))
        ew = ep(tc.tile_pool(name="ew", bufs=2))
        smp = ep(tc.tile_pool(name="smp", bufs=1))
        ps_m = ep(tc.tile_pool(name="ps_m", bufs=2, space="PSUM"))
        ps_d = ep(tc.tile_pool(name="ps_d", bufs=2, space="PSUM"))
        ps1 = ep(tc.tile_pool(name="ps1", bufs=2, space="PSUM"))
        ps_t = ep(tc.tile_pool(name="ps_t", bufs=2, space="PSUM"))

        def inproj_scan(c, s):
            zp = ps1.tile([128, 4 * GW[s]], F32, tag="ps", name="zp")
            for k in range(KCH[s]):
                nc.tensor.matmul(
                    zp[:],
                    sb[f"xt{s}"][:, k * TB + c * 128 : k * TB + (c + 1) * 128],
                    sb[f"wih{s}"][:, k * 4 * GW[s] : (k + 1) * 4 * GW[s]],
                    start=(k == 0),
                    stop=(k == KCH[s] - 1),
                )
            z = zsb.tile([128, 4 * GW[s]], BF16, tag=f"z{s}", name="z")
            if s == 2:
                nc.vector.tensor_add(z[:], zp[:], sb["bias2t"][:])
            elif s == 1:
                nc.scalar.copy(z[:], zp[:])
            else:
                nc.vector.tensor_copy(z[:], zp[:])
            return z

        def mod_step(t, zt):
            trel = t % TC
            gp = ps_m.tile([96, 4 * GP], F32, tag="gm", name="gp")
            for s in range(3):
                if GW[s] == GP:
                    gout = gp[32 * s : 32 * s + 32, :]
                else:
                    gout = gp[32 * s : 32 * s + 32, :].rearrange(
                        "p (g x) -> p g x", x=GP
                    )[:, :, 0 : GW[s]]
                nc.tensor.matmul(
                    gout,
                    sb["i32s"][32 * trel : 32 * trel + 32, :],
                    zt[s][32 * trel : 32 * trel + 32, :],
                    start=True,
                    stop=False,
                    tile_position=(32 * trel, 32 * s),
                )
                nc.tensor.matmul(
                    gout,
                    hmt[0 : HID[s], t * 96 + 32 * s : t * 96 + 32 * s + 32],
                    sb[f"whh{s}"][0 : HID[s], :],
                    start=False,
                    stop=True,
                    tile_position=(0, 32 * s),
                )
            # gates: (i | f | o | 2g); one sigmoid covers all four
            sg = ew.tile([96, 4 * GP], BF16, tag="sg", name="sg")
            nc.scalar.activation(sg[:], gp[:], AF.Sigmoid)
            g2 = ew.tile([96, GP], BF16, tag="g2", name="g2")  # 2*tanh(g)
            nc.vector.tensor_scalar(g2[:], sg[:, 3 * GP : 4 * GP], 4.0, -2.0, ALU.mult, ALU.add)
            m1 = ew.tile([96, GP], BF16, tag="m1", name="m1")
            nc.vector.tensor_mul(m1[:], sg[:, GP : 2 * GP], c3[:])  # f * C
            m2 = ew.tile([96, GP], BF16, tag="m2", name="m2")
            nc.gpsimd.tensor_mul(m2[:], sg[:, 0:GP], g2[:])  # i * 2tanh(g)
            nc.vector.tensor_add(c3[:], m1[:], m2[:])  # C' = 2c'
            sc = ew.tile([96, GP], BF16, tag="sc", name="sc")
            nc.scalar.activation(sc[:], c3[:], AF.Sigmoid)
            tc2 = ew.tile([96, GP], BF16, tag="tc2", name="tc2")  # 2*tanh(c)
            nc.vector.tensor_scalar(tc2[:], sc[:], 4.0, -2.0, ALU.mult, ALU.add)
            h2d = ew.tile([96, GP], BF16, tag="h2d", name="h2d")  # 2*h2
            nc.vector.tensor_mul(h2d[:], sg[:, 2 * GP : 3 * GP], tc2[:])
            pt = ps_t.tile([128, 160], BF16, tag="tr", name="pt")
            nc.tensor.transpose(pt[:, 0:96], h2d[:], sb["idbf"][0:96, 0:96])
            se = ew.tile([128, 96], BF16, tag="se", name="se")
            nc.scalar.activation(se[:], pt[:, 0:96], AF.Sigmoid)  # sigma(2*h2)
            nc.vector.tensor_scalar(
                hmt[:, (t + 1) * 96 : (t + 2) * 96], se[:], 2.0, -1.0, ALU.mult, ALU.add
            )  # tanh(h2)

        def fc_piece(c, s):
            fp = ps1.tile([FCD[s], 128], F32, tag="ps", name="fp")
            nc.tensor.matmul(
                fp[:],
                sb[f"fcw{s}"][:],
                hmt_b[0 : HID[s], c * TC + 1 : c * TC + 5, 32 * s : 32 * s + 32],
                start=True,
                stop=True,
            )
            ft = fpool.tile([FCD[s], 128], BF16, tag=f"ft{s}", name="ft")
            nc.scalar.activation(ft[:], fp[:], AF.Tanh, bias=sb[f"fcb{s}"][:])
            return ft

        def zd_half(fts, zd, h):
            zdp = ps1.tile([128, 512], F32, tag="ps", name="zdp")
            sl = slice(512 * h, 512 * (h + 1))
            for s in range(3):
                nc.tensor.matmul(
                    zdp[:], fts[s][:], sb[f"wihd{s}"][:, sl],
                    start=(s == 0), stop=(s == 2),
                )
            nc.vector.tensor_add(zd[:, sl], zdp[:], sb["bdrow2"][:, sl])

        def dial_step(t, zd):
            trel = t % TC
            # one bank [64, 512]: rows 0-31 = (i|f), rows 32-63 = (o|2g)
            gd = ps_d.tile([64, 512], F32, tag="gd", name="gd")
            for hh, base in ((0, 0), (1, 32)):
                sl = slice(512 * hh, 512 * (hh + 1))
                nc.tensor.matmul(
                    gd[base : base + 32, :],
                    sb["i32s"][32 * trel : 32 * trel + 32, :],
                    zd[32 * trel : 32 * trel + 32, sl],
                    start=True,
                    stop=False,
                    tile_position=(32 * trel, base),
                )
                for k in range(2):
                    nc.tensor.matmul(
                        gd[base : base + 32, :],
                        hdt[:, t * 64 + 32 * k : t * 64 + 32 * k + 32],
                        sb[f"whhd{k}"][:, sl],
                        start=False,
                        stop=(k == 1),
                        tile_position=(0, base),
                    )
            sgd = ew.tile([64, 512], BF16, tag="sgd", name="sgd")
            nc.scalar.activation(sgd[:], gd[:], AF.Sigmoid)
            g2d = ew.tile([32, DH], BF16, tag="g2d", name="g2d")  # 2*tanh(g), base 0
            nc.vector.tensor_scalar(g2d[:], sgd[32:64, DH : 2 * DH], 4.0, -2.0, ALU.mult, ALU.add)
            m2 = ew.tile([32, DH], BF16, tag="m2d", name="m2d")
            nc.vector.tensor_mul(m2[:], sgd[0:32, 0:DH], g2d[:])  # i * 2tanh(g)
            m1 = ew.tile([32, DH], BF16, tag="m1d", name="m1d")
            nc.gpsimd.tensor_mul(m1[:], sgd[0:32, DH : 2 * DH], cd[:])  # f * C
            nc.vector.tensor_add(cd[:], m1[:], m2[:])
            scd = ew.tile([32, DH], BF16, tag="scd", name="scd")
            nc.scalar.activation(scd[:], cd[:], AF.Sigmoid)
            tc64 = ew.tile([64, DH], BF16, tag="tc64", name="tc64")
            nc.vector.tensor_scalar(tc64[32:64, :], scd[:], 2.0, -1.0, ALU.mult, ALU.add)
            h2 = ew.tile([32, DH], BF16, tag="h2", name="h2")
            nc.vector.tensor_mul(h2[:], sgd[32:64, 0:DH], tc64[32:64, :])  # o * tanh(c)
            ptd = ps_t.tile([128, 160], BF16, tag="tr", name="ptd")
            for k in range(2):
                nc.tensor.transpose(
                    ptd[:, 32 * k : 32 * (k + 1)],
                    h2[:, 128 * k : 128 * (k + 1)],
                    sb["idbf"][0:32, 0:32],
                )
            nc.vector.tensor_copy(hdt[:, (t + 1) * 64 : (t + 2) * 64], ptd[:, 0:64])

        def head(c):
            hp = ps1.tile([DF, 128], F32, tag="ps", name="hp")
            for k in range(2):
                nc.tensor.matmul(
                    hp[:],
                    sb[f"fcoutw{k}"][:],
                    hdt_b[:, c * TC + 1 : c * TC + 5, 32 * k : 32 * k + 32],
                    start=(k == 0),
                    stop=(k == 1),
                )
            hst = fpool.tile([DF, 128], BF16, tag="hst", name="hst")
            nc.scalar.activation(hst[:], hp[:], AF.Tanh, bias=sb["fcoutb"][:])
            lp = ps1.tile([128, NCLS], F32, tag="ps", name="lp")
            nc.tensor.matmul(lp[:], hst[:], sb["smaxwt"][:], start=True, stop=True)
            nc.vector.tensor_add(lgt[:, NCLS * c : NCLS * (c + 1)], lp[:], sb["smaxbt"][:])

        def endpass():
            lg3 = lgt[:].rearrange("p (c j) -> p c j", j=NCLS)
            mx = smp.tile([128, NCH], F32, tag="mx", name="mx")
            nc.vector.tensor_reduce(mx[:].unsqueeze(2), lg3, mybir.AxisListType.X, ALU.max)
            mxb = mx[:].unsqueeze(2).broadcast_to([128, NCH, NCLS])
            lc = smp.tile([128, NCH * NCLS], F32, tag="lc", name="lc")
            lc3 = lc[:].rearrange("p (c j) -> p c j", j=NCLS)
            nc.vector.tensor_sub(lc3, lg3, mxb)
            ex = smp.tile([128, NCH * NCLS], F32, tag="ex", name="ex")
            nc.scalar.activation(ex[:], lc[:], AF.Exp)
            se = smp.tile([128, NCH], F32, tag="sme", name="sme")
            nc.vector.tensor_reduce(
                se[:].unsqueeze(2), ex[:].rearrange("p (c j) -> p c j", j=NCLS),
                mybir.AxisListType.X, ALU.add,
            )
            lns = smp.tile([128, NCH], F32, tag="lns", name="lns")
            nc.scalar.activation(lns[:], se[:], AF.Ln)
            fin = smp.tile([128, NCH * NCLS], F32, tag="fin", name="fin")
            nc.vector.tensor_sub(
                fin[:].rearrange("p (c j) -> p c j", j=NCLS),
                lc3,
                lns[:].unsqueeze(2).broadcast_to([128, NCH, NCLS]),
            )
            for c in range(NCH):
                nc.sync.dma_start(
                    out[:, c * TC : (c + 1) * TC, :].rearrange("i t c -> t i c"),
                    fin[:, NCLS * c : NCLS * (c + 1)],
                )

        hmt_b = hmt[:].rearrange("p (t g) -> p t g", g=96)
        hdt_b = hdt[:].rearrange("p (t g) -> p t g", g=64)

        # software-pipelined: inproj runs 1 chunk ahead, dialogue lags 2 chunks,
        # bulk matmul groups woven between scan steps to keep PE streaming.
        ztcur = [inproj_scan(0, s) for s in range(3)]
        ztnext = [None] * 3
        fts = [None] * 3
        zdd = zdw = None
        for c in range(NCH + 3):
            for trel in range(TC):
                if c < NCH:
                    mod_step(c * TC + trel, ztcur)
                if c + 1 < NCH and trel < 3:
                    ztnext[trel] = inproj_scan(c + 1, trel)
                if 2 <= c < NCH + 2:
                    dial_step((c - 2) * TC + trel, zdd)
                if 1 <= c <= NCH:
                    if trel == 0:
                        fts[0] = fc_piece(c - 1, 0)
                        fts[1] = fc_piece(c - 1, 1)
                    elif trel == 1:
                        fts[2] = fc_piece(c - 1, 2)
                        zdw = zdpool.tile([128, 4 * DH], BF16, tag="zd", name="zd")
                        zd_half(fts, zdw, 0)
                    elif trel == 2:
                        zd_half(fts, zdw, 1)
                if 3 <= c < NCH + 3 and trel == 3:
                    head(c - 3)
            ztcur, ztnext = ztnext, [None] * 3
            zdd = zdw
        endpass()

    nc.compile()
    _CACHE["nc"] = nc
    return nc


def _prep_core(inputs, core):
    """Build the per-core input map (host-side shard/transpose/pad/bf16)."""
    d = {}
    sl = slice(core * BSH, (core + 1) * BSH)
    for s in range(3):
        D = IN_DIMS[s]
        shard = np.asarray(inputs[f"mod{s}"][sl], np.float32)  # [32, T, D]
        xts = np.zeros((DPAD[s], TB), np.float32)
        xts[:D] = shard.transpose(2, 1, 0).reshape(D, TB)
        gw = GW[s]
        wt = np.zeros((DPAD[s], 4 * gw), np.float32)
        wt[:D] = _gate_reorder_T(np.asarray(inputs[f"w_ih{s}"], np.float32), HID[s], gw)
        bias = _gate_reorder_b_w(
            np.asarray(inputs[f"b_ih{s}"], np.float32)
            + np.asarray(inputs[f"b_hh{s}"], np.float32),
            HID[s],
            gw,
        )
        if s == 2:
            d["bias2t"] = np.broadcast_to(bias, (128, 4 * gw)).copy()
        else:
            xts[D] = 1.0
            wt[D] = bias
        d[f"xt{s}"] = _bf16(xts)
        d[f"wih{s}"] = _bf16(wt)
        d[f"whh{s}"] = _bf16(
            _gate_reorder_T(np.asarray(inputs[f"w_hh{s}"], np.float32), HID[s], gw)
        )
        d[f"fcw{s}"] = _bf16(np.asarray(inputs[f"fc_w{s}"], np.float32).T)
        d[f"fcb{s}"] = np.asarray(inputs[f"fc_b{s}"], np.float32).reshape(-1, 1).copy()
    wihdt = _gate_reorder_T(np.asarray(inputs["w_ih_d"], np.float32), DH, DH)  # [250, 1024]
    d["wihd0"] = _bf16(wihdt[0:100])
    d["wihd1"] = _bf16(wihdt[100:150])
    d["wihd2"] = _bf16(wihdt[150:250])
    bd = _gate_reorder_b(
        np.asarray(inputs["b_ih_d"], np.float32)
        + np.asarray(inputs["b_hh_d"], np.float32),
        DH,
        DH,
    )
    d["bdrow2"] = np.broadcast_to(bd, (128, 4 * DH)).copy()
    whhdt = _gate_reorder_T(np.asarray(inputs["w_hh_d"], np.float32), DH, DH)  # [256, 1024]
    d["whhd0"] = _bf16(whhdt[0:128])
    d["whhd1"] = _bf16(whhdt[128:256])
    fow = np.asarray(inputs["fc_out_w"], np.float32).T  # [256, 128]
    d["fcoutw0"] = _bf16(fow[0:128])
    d["fcoutw1"] = _bf16(fow[128:256])
    d["fcoutb"] = np.asarray(inputs["fc_out_b"], np.float32).reshape(-1, 1).copy()
    d["smaxwt"] = _bf16(np.asarray(inputs["smax_w"], np.float32).T)
    d["smaxbt"] = np.broadcast_to(
        np.asarray(inputs["smax_b"], np.float32), (128, NCLS)
    ).copy()
    d["idbf"] = _bf16(np.eye(128, dtype=np.float32))
    i32 = np.zeros((128, 32), np.float32)
    for k in range(4):
        i32[32 * k : 32 * (k + 1)] = np.eye(32)
    d["i32s"] = _bf16(i32)
    return d


def run(inputs, trace=False, **kw):
    nc = _build()
    in_maps = [_prep_core(inputs, i) for i in range(NCORES)]
    res = run_bass_kernel_spmd(nc, in_maps, list(range(NCORES)), trace=trace, **kw)
    full = np.concatenate(
        [np.asarray(res.results[i]["out"], np.float32) for i in range(NCORES)], axis=0
    )
    return full, res


def kernel(**inputs) -> np.ndarray:
    out, _ = run(inputs, trace=False)
    return out


# revision 13
# speedup vs baseline: 1.2253x; 1.2046x over previous
"""BC-LSTM Trainium2 kernel: data-parallel over batch on 8 NeuronCores.

Shapes (hardcoded): B=256, T=128, IN_DIMS=[300,100,512], HID=[128,64,128],
FC=[100,50,100], DH=256, DF=128, NC=6. Per-core batch shard b=32.

v2 layout strategy (per core):
- Host pre-transposes activations/weights; g-gate columns pre-scaled x2 so
  tanh(x) = 2*sigmoid(2x)-1 turns every scan activation into ONE sigmoid
  table-set (no ACT table thrash, fewer ACT ops).
- Cell state carried as C = 2c so tanh(c) = 2*sigmoid(C)-1 stays one sigmoid.
- Input projections Z computed in bulk [128(4t x 32b), 4H] chunks on PE,
  evacuated PSUM->SBUF split across DVE/ACT.
- Modality scans packed on partitions 0..95; gates [96,512] from identity
  Z-gather + h-stationary matmuls at distinct PE col bands (concurrent).
- h fed back via PE transpose (no DMA transpose); the h-tanh is fused into
  the PSUM evacuation as sigmoid + (2x-1) tensor_scalar.
- Dialogue gates packed [64,512] one PSUM bank: rows 0-31 = (i|f), rows
  32-63 = (o|g) -> ONE sigmoid per step.
- log_softmax deferred to one end-pass (Exp/Ln tables loaded once).
"""

import sys

sys.path.insert(0, "/opt/trn_rl_repo")

import numpy as np
import ml_dtypes

import concourse.bass as bass
import concourse.tile as tile
from concourse import bacc, mybir
from concourse.bass_utils import run_bass_kernel_spmd

F32 = mybir.dt.float32
BF16 = mybir.dt.bfloat16
AF = mybir.ActivationFunctionType
ALU = mybir.AluOpType

NCORES = 8
B, T = 256, 128
BSH = B // NCORES  # 32
TB = T * BSH  # 4096
IN_DIMS = [300, 100, 512]
HID = [128, 64, 128]
FCD = [100, 50, 100]
DH, DF, NCLS = 256, 128, 6
GP = 128  # per-gate padded width for modality scans
NCH = 32  # chunks
TC = 4  # timesteps per chunk (TC*BSH = 128 rows)

DPAD = [384, 128, 512]  # mod0: 300+bias_row+pad, mod1: 100+bias_row+pad, mod2: exact
KCH = [3, 1, 4]  # number of 128-row K chunks
GW = [128, 64, 128]  # per-gate column width of the packed Z/gate matmuls


def _gate_reorder_T(w, H, P):
    """w [4H, D] torch gate order (i,f,g,o) -> W.T [D, 4P] order (i,f,o,g),
    each gate padded to P columns. The g gate is scaled x2 (tanh-via-sigmoid)."""
    D = w.shape[1]
    out = np.zeros((D, 4 * P), np.float32)
    for gi, src in enumerate([0, 1, 3, 2]):
        out[:, gi * P : gi * P + H] = w[src * H : (src + 1) * H, :].T
    out[:, 3 * P : 4 * P] *= 2.0
    return out


def _gate_reorder_b_w(bvec, H, P):
    out = np.zeros(4 * P, np.float32)
    for gi, src in enumerate([0, 1, 3, 2]):
        out[gi * P : gi * P + H] = bvec[src * H : (src + 1) * H]
    out[3 * P : 4 * P] *= 2.0
    return out


def _gate_reorder_b(bvec, H, P):
    out = np.zeros(4 * P, np.float32)
    for gi, src in enumerate([0, 1, 3, 2]):
        out[gi * P : gi * P + H] = bvec[src * H : (src + 1) * H]
    out[3 * P : 4 * P] *= 2.0
    return out


def _bf16(x):
    return np.ascontiguousarray(x).astype(ml_dtypes.bfloat16)


_CACHE = {}


def _build():
    if "nc" in _CACHE:
        return _CACHE["nc"]
    nc = bacc.Bacc("TRN2", target_bir_lowering=False, debug=False, num_devices=NCORES)

    def din(name, shape, dt=BF16):
        return nc.dram_tensor(name, shape, dt, kind="ExternalInput").ap()

    # per-core inputs
    xt = [din(f"xt{s}", [DPAD[s], TB]) for s in range(3)]
    wih = [din(f"wih{s}", [DPAD[s], 4 * GW[s]]) for s in range(3)]
    bias2t = din("bias2t", [128, 4 * GP], F32)
    whh = [din(f"whh{s}", [HID[s], 4 * GW[s]]) for s in range(3)]
    fcw = [din(f"fcw{s}", [HID[s], FCD[s]]) for s in range(3)]
    fcb = [din(f"fcb{s}", [FCD[s], 1], F32) for s in range(3)]
    wihd = [din(f"wihd{s}", [FCD[s], 4 * DH]) for s in range(3)]
    bdrow2 = din("bdrow2", [128, 4 * DH], F32)
    whhd = [din(f"whhd{k}", [128, 4 * DH]) for k in range(2)]
    fcoutw = [din(f"fcoutw{k}", [128, DF]) for k in range(2)]
    fcoutb = din("fcoutb", [DF, 1], F32)
    smaxwt = din("smaxwt", [DF, NCLS])
    smaxbt = din("smaxbt", [128, NCLS], F32)
    idbf = din("idbf", [128, 128])
    i32s = din("i32s", [128, 32])
    out = nc.dram_tensor("out", [BSH, T, NCLS], F32, kind="ExternalOutput").ap()

    with tile.TileContext(nc) as tc, bass.ExitStack() as ctx:
        ep = ctx.enter_context
        stat = ep(tc.tile_pool(name="stat", bufs=1))
        sb = {}
        # small weights first (so the big xt loads don't delay them)
        for s in range(3):
            sb[f"wih{s}"] = stat.tile([128, KCH[s] * 4 * GW[s]], BF16, tag=f"wih{s}", name=f"wih{s}")
            for k in range(KCH[s]):
                nc.sync.dma_start(
                    sb[f"wih{s}"][:, k * 4 * GW[s] : (k + 1) * 4 * GW[s]],
                    wih[s][k * 128 : (k + 1) * 128, :],
                )
            sb[f"whh{s}"] = stat.tile([HID[s], 4 * GW[s]], BF16, tag=f"whh{s}", name=f"whh{s}")
            nc.sync.dma_start(sb[f"whh{s}"][:], whh[s][:])
            sb[f"fcw{s}"] = stat.tile([HID[s], FCD[s]], BF16, tag=f"fcw{s}", name=f"fcw{s}")
            nc.sync.dma_start(sb[f"fcw{s}"][:], fcw[s][:])
            sb[f"fcb{s}"] = stat.tile([FCD[s], 1], F32, tag=f"fcb{s}", name=f"fcb{s}")
            nc.sync.dma_start(sb[f"fcb{s}"][:], fcb[s][:])
            sb[f"wihd{s}"] = stat.tile([FCD[s], 4 * DH], BF16, tag=f"wihd{s}", name=f"wihd{s}")
            nc.sync.dma_start(sb[f"wihd{s}"][:], wihd[s][:])
        for name, src, shp, dt in [
            ("bias2t", bias2t, [128, 4 * GP], F32),
            ("bdrow2", bdrow2, [128, 4 * DH], F32),
            ("fcoutb", fcoutb, [DF, 1], F32),
            ("smaxwt", smaxwt, [DF, NCLS], BF16),
            ("smaxbt", smaxbt, [128, NCLS], F32),
            ("idbf", idbf, [128, 128], BF16),
            ("i32s", i32s, [128, 32], BF16),
        ]:
            sb[name] = stat.tile(shp, dt, tag=name, name=name)
            nc.sync.dma_start(sb[name][:], src[:])
        for k in range(2):
            sb[f"whhd{k}"] = stat.tile([128, 4 * DH], BF16, tag=f"whhd{k}", name=f"whhd{k}")
            nc.sync.dma_start(sb[f"whhd{k}"][:], whhd[k][:])
            sb[f"fcoutw{k}"] = stat.tile([128, DF], BF16, tag=f"fcoutw{k}", name=f"fcoutw{k}")
            nc.sync.dma_start(sb[f"fcoutw{k}"][:], fcoutw[k][:])
        # big activation loads, split by column quarter so chunk 0 starts early
        for s in range(3):
            sb[f"xt{s}"] = stat.tile([128, KCH[s] * TB], BF16, tag=f"xt{s}", name=f"xt{s}")
        QW = TB // 4
        for q in range(4):
            for s in range(3):
                for k in range(KCH[s]):
                    nc.sync.dma_start(
                        sb[f"xt{s}"][:, k * TB + q * QW : k * TB + (q + 1) * QW],
                        xt[s][k * 128 : (k + 1) * 128, q * QW : (q + 1) * QW],
                    )

        # history buffers (block t holds state BEFORE step t; block t+1 = output of step t)
        hmt = stat.tile([128, (T + 1) * 96], BF16, tag="hmt")
        hdt = stat.tile([128, (T + 1) * 64], BF16, tag="hdt")
        c3 = stat.tile([96, GP], BF16, tag="c3")  # C = 2c for the 3 mod scans
        cd = stat.tile([32, DH], BF16, tag="cd")  # C = 2c for the dialogue scan
        lgt = stat.tile([128, NCH * NCLS], F32, tag="lgt")  # logits staging
        nc.vector.memset(hmt[:, 0:96], 0.0)
        nc.vector.memset(hdt[:, 0:64], 0.0)
        nc.vector.memset(c3[:], 0.0)
        nc.vector.memset(cd[:], 0.0)

        zsb = ep(tc.tile_pool(name="zsb", bufs=2))
        zre = ep(tc.tile_pool(name="zre", bufs=2))
        zdpool = ep(tc.tile_pool(name="zdpool", bufs=2))
        fpool = ep(tc.tile_pool(name="fpool", bufs=2))
        ew = ep(tc.tile_pool(name="ew", bufs=3))
        smp = ep(tc.tile_pool(name="smp", bufs=1))
        ps_m = ep(tc.tile_pool(name="ps_m", bufs=2, space="PSUM"))
        ps_d = ep(tc.tile_pool(name="ps_d", bufs=2, space="PSUM"))
        ps1 = ep(tc.tile_pool(name="ps1", bufs=2, space="PSUM"))
        ps_t = ep(tc.tile_pool(name="ps_t", bufs=2, space="PSUM"))

        def inproj_scan(c, s):
            zp = ps1.tile([128, 4 * GW[s]], F32, tag="ps", name="zp")
            for k in range(KCH[s]):
                nc.tensor.matmul(
                    zp[:],
                    sb[f"xt{s}"][:, k * TB + c * 128 : k * TB + (c + 1) * 128],
                    sb[f"wih{s}"][:, k * 4 * GW[s] : (k + 1) * 4 * GW[s]],
                    start=(k == 0),
                    stop=(k == KCH[s] - 1),
                )
            z = zsb.tile([128, 4 * GW[s]], BF16, tag=f"z{s}", name="z")
            if s == 2:
                nc.vector.tensor_add(z[:], zp[:], sb["bias2t"][:])
            elif s == 1:
                nc.scalar.copy(z[:], zp[:])
            else:
                nc.vector.tensor_copy(z[:], zp[:])
            return z

        def gather_mod(zt):
            """DMA-scatter the chunk's Z from [4t x 32b, gates] into scan-packed
            rows [96, t-blocks of 512] so ONE identity matmul injects all scans."""
            zm = zre.tile([96, TC * 4 * GP], BF16, tag="zrem", name="zm")
            # zero mod1's pad columns: the shared identity matmul streams them,
            # and 0 * NaN = NaN would poison every output row of the inject
            nc.vector.memset(
                zm[32:64, :].rearrange("p (g x) -> p g x", x=GP)[:, :, GW[1] : GP], 0.0
            )
            for s in range(3):
                for t in range(TC):
                    if GW[s] == GP:
                        nc.sync.dma_start(
                            zm[32 * s : 32 * s + 32, t * 4 * GP : (t + 1) * 4 * GP],
                            zt[s][32 * t : 32 * t + 32, :],
                        )
                    else:
                        nc.sync.dma_start(
                            zm[
                                32 * s : 32 * s + 32, t * 4 * GP : (t + 1) * 4 * GP
                            ].rearrange("p (g x) -> p g x", x=GP)[:, :, 0 : GW[s]],
                            zt[s][32 * t : 32 * t + 32, :].rearrange(
                                "p (g x) -> p g x", x=GW[s]
                            ),
                        )
            return zm

        def mod_step(t, zm):
            trel = t % TC
            gp = ps_m.tile([96, 4 * GP], F32, tag="gm", name="gp")
            nc.tensor.matmul(
                gp[:],
                sb["idbf"][0:96, 0:96],
                zm[:, trel * 4 * GP : (trel + 1) * 4 * GP],
                start=True,
                stop=False,
            )
            for s in range(3):
                if GW[s] == GP:
                    gout = gp[32 * s : 32 * s + 32, :]
                else:
                    gout = gp[32 * s : 32 * s + 32, :].rearrange(
                        "p (g x) -> p g x", x=GP
                    )[:, :, 0 : GW[s]]
                nc.tensor.matmul(
                    gout,
                    hmt[0 : HID[s], t * 96 + 32 * s : t * 96 + 32 * s + 32],
                    sb[f"whh{s}"][0 : HID[s], :],
                    start=False,
                    stop=(s == 2),
                    tile_position=(0, 32 * s),
                )
            # gates: (i | f | o | 2g); one sigmoid covers all four
            sg = ew.tile([96, 4 * GP], BF16, tag="sg", name="sg")
            nc.scalar.activation(sg[:], gp[:], AF.Sigmoid)
            g2 = ew.tile([96, GP], BF16, tag="g2", name="g2")  # 2*tanh(g)
            nc.vector.tensor_scalar(g2[:], sg[:, 3 * GP : 4 * GP], 4.0, -2.0, ALU.mult, ALU.add)
            m1 = ew.tile([96, GP], BF16, tag="m1", name="m1")
            nc.vector.tensor_mul(m1[:], sg[:, GP : 2 * GP], c3[:])  # f * C
            m2 = ew.tile([96, GP], BF16, tag="m2", name="m2")
            nc.gpsimd.tensor_mul(m2[:], sg[:, 0:GP], g2[:])  # i * 2tanh(g)
            nc.vector.tensor_add(c3[:], m1[:], m2[:])  # C' = 2c'
            sc = ew.tile([96, GP], BF16, tag="sc", name="sc")
            nc.scalar.activation(sc[:], c3[:], AF.Sigmoid)
            tc2 = ew.tile([96, GP], BF16, tag="tc2", name="tc2")  # 2*tanh(c)
            nc.vector.tensor_scalar(tc2[:], sc[:], 4.0, -2.0, ALU.mult, ALU.add)
            h2d = ew.tile([96, GP], BF16, tag="h2d", name="h2d")  # 2*h2
            nc.vector.tensor_mul(h2d[:], sg[:, 2 * GP : 3 * GP], tc2[:])
            pt = ps_t.tile([128, 160], BF16, tag="tr", name="pt")
            nc.tensor.transpose(pt[:, 0:96], h2d[:], sb["idbf"][0:96, 0:96])
            se = ew.tile([128, 96], BF16, tag="se", name="se")
            nc.scalar.activation(se[:], pt[:, 0:96], AF.Sigmoid)  # sigma(2*h2)
            nc.vector.tensor_scalar(
                hmt[:, (t + 1) * 96 : (t + 2) * 96], se[:], 2.0, -1.0, ALU.mult, ALU.add
            )  # tanh(h2)

        def fc_piece(c, s):
            fp = ps1.tile([FCD[s], 128], F32, tag="ps", name="fp")
            nc.tensor.matmul(
                fp[:],
                sb[f"fcw{s}"][:],
                hmt_b[0 : HID[s], c * TC + 1 : c * TC + 5, 32 * s : 32 * s + 32],
                start=True,
                stop=True,
            )
            ft = fpool.tile([FCD[s], 128], BF16, tag=f"ft{s}", name="ft")
            nc.scalar.activation(ft[:], fp[:], AF.Tanh, bias=sb[f"fcb{s}"][:])
            return ft

        def zd_half(fts, zd, h):
            zdp = ps1.tile([128, 512], F32, tag="ps", name="zdp")
            sl = slice(512 * h, 512 * (h + 1))
            for s in range(3):
                nc.tensor.matmul(
                    zdp[:], fts[s][:], sb[f"wihd{s}"][:, sl],
                    start=(s == 0), stop=(s == 2),
                )
            nc.vector.tensor_add(zd[:, sl], zdp[:], sb["bdrow2"][:, sl])

        def gather_dial(zd):
            zg = zre.tile([64, TC * 512], BF16, tag="zred", name="zg")
            for hh in range(2):
                for t in range(TC):
                    nc.sync.dma_start(
                        zg[32 * hh : 32 * hh + 32, t * 512 : (t + 1) * 512],
                        zd[32 * t : 32 * t + 32, 512 * hh : 512 * (hh + 1)],
                    )
            return zg

        def dial_step(t, zg):
            trel = t % TC
            # one bank [64, 512]: rows 0-31 = (i|f), rows 32-63 = (o|2g)
            gd = ps_d.tile([64, 512], F32, tag="gd", name="gd")
            nc.tensor.matmul(
                gd[:],
                sb["idbf"][0:64, 0:64],
                zg[:, trel * 512 : (trel + 1) * 512],
                start=True,
                stop=False,
            )
            for hh, base in ((0, 0), (1, 32)):
                sl = slice(512 * hh, 512 * (hh + 1))
                for k in range(2):
                    nc.tensor.matmul(
                        gd[base : base + 32, :],
                        hdt[:, t * 64 + 32 * k : t * 64 + 32 * k + 32],
                        sb[f"whhd{k}"][:, sl],
                        start=False,
                        stop=(k == 1),
                        tile_position=(0, base),
                    )
            sgd = ew.tile([64, 512], BF16, tag="sgd", name="sgd")
            nc.scalar.activation(sgd[:], gd[:], AF.Sigmoid)
            g2d = ew.tile([32, DH], BF16, tag="g2d", name="g2d")  # 2*tanh(g), base 0
            nc.vector.tensor_scalar(g2d[:], sgd[32:64, DH : 2 * DH], 4.0, -2.0, ALU.mult, ALU.add)
            m2 = ew.tile([32, DH], BF16, tag="m2d", name="m2d")
            nc.vector.tensor_mul(m2[:], sgd[0:32, 0:DH], g2d[:])  # i * 2tanh(g)
            m1 = ew.tile([32, DH], BF16, tag="m1d", name="m1d")
            nc.gpsimd.tensor_mul(m1[:], sgd[0:32, DH : 2 * DH], cd[:])  # f * C
            nc.vector.tensor_add(cd[:], m1[:], m2[:])
            scd = ew.tile([32, DH], BF16, tag="scd", name="scd")
            nc.scalar.activation(scd[:], cd[:], AF.Sigmoid)
            tc64 = ew.tile([64, DH], BF16, tag="tc64", name="tc64")
            nc.vector.tensor_scalar(tc64[32:64, :], scd[:], 2.0, -1.0, ALU.mult, ALU.add)
            h2 = ew.tile([32, DH], BF16, tag="h2", name="h2")
            nc.vector.tensor_mul(h2[:], sgd[32:64, 0:DH], tc64[32:64, :])  # o * tanh(c)
            ptd = ps_t.tile([128, 160], BF16, tag="tr", name="ptd")
            for k in range(2):
                nc.tensor.transpose(
                    ptd[:, 32 * k : 32 * (k + 1)],
                    h2[:, 128 * k : 128 * (k + 1)],
                    sb["idbf"][0:32, 0:32],
                )
            nc.vector.tensor_copy(hdt[:, (t + 1) * 64 : (t + 2) * 64], ptd[:, 0:64])

        def head(c):
            hp = ps1.tile([DF, 128], F32, tag="ps", name="hp")
            for k in range(2):
                nc.tensor.matmul(
                    hp[:],
                    sb[f"fcoutw{k}"][:],
                    hdt_b[:, c * TC + 1 : c * TC + 5, 32 * k : 32 * k + 32],
                    start=(k == 0),
                    stop=(k == 1),
                )
            hst = fpool.tile([DF, 128], BF16, tag="hst", name="hst")
            nc.scalar.activation(hst[:], hp[:], AF.Tanh, bias=sb["fcoutb"][:])
            lp = ps1.tile([128, NCLS], F32, tag="ps", name="lp")
            nc.tensor.matmul(lp[:], hst[:], sb["smaxwt"][:], start=True, stop=True)
            nc.vector.tensor_add(lgt[:, NCLS * c : NCLS * (c + 1)], lp[:], sb["smaxbt"][:])

        def endpass():
            lg3 = lgt[:].rearrange("p (c j) -> p c j", j=NCLS)
            mx = smp.tile([128, NCH], F32, tag="mx", name="mx")
            nc.vector.tensor_reduce(mx[:].unsqueeze(2), lg3, mybir.AxisListType.X, ALU.max)
            mxb = mx[:].unsqueeze(2).broadcast_to([128, NCH, NCLS])
            lc = smp.tile([128, NCH * NCLS], F32, tag="lc", name="lc")
            lc3 = lc[:].rearrange("p (c j) -> p c j", j=NCLS)
            nc.vector.tensor_sub(lc3, lg3, mxb)
            ex = smp.tile([128, NCH * NCLS], F32, tag="ex", name="ex")
            nc.scalar.activation(ex[:], lc[:], AF.Exp)
            se = smp.tile([128, NCH], F32, tag="sme", name="sme")
            nc.vector.tensor_reduce(
                se[:].unsqueeze(2), ex[:].rearrange("p (c j) -> p c j", j=NCLS),
                mybir.AxisListType.X, ALU.add,
            )
            lns = smp.tile([128, NCH], F32, tag="lns", name="lns")
            nc.scalar.activation(lns[:], se[:], AF.Ln)
            fin = smp.tile([128, NCH * NCLS], F32, tag="fin", name="fin")
            nc.vector.tensor_sub(
                fin[:].rearrange("p (c j) -> p c j", j=NCLS),
                lc3,
                lns[:].unsqueeze(2).broadcast_to([128, NCH, NCLS]),
            )
            for c in range(NCH):
                nc.sync.dma_start(
                    out[:, c * TC : (c + 1) * TC, :].rearrange("i t c -> t i c"),
                    fin[:, NCLS * c : NCLS * (c + 1)],
                )

        hmt_b = hmt[:].rearrange("p (t g) -> p t g", g=96)
        hdt_b = hdt[:].rearrange("p (t g) -> p t g", g=64)

        # software-pipelined: inproj runs 1 chunk ahead, dialogue lags 2 chunks,
        # bulk matmul groups woven between scan steps to keep PE streaming.
        zt0 = [inproj_scan(0, s) for s in range(3)]
        zmcur = gather_mod(zt0)
        ztnext = [None] * 3
        fts = [None] * 3
        zgd = zgw = zdw = None
        for c in range(NCH + 3):
            for trel in range(TC):
                if c < NCH:
                    mod_step(c * TC + trel, zmcur)
                if c + 1 < NCH and trel < 3:
                    ztnext[trel] = inproj_scan(c + 1, trel)
                if 2 <= c < NCH + 2:
                    dial_step((c - 2) * TC + trel, zgd)
                if 1 <= c <= NCH:
                    if trel == 0:
                        fts[0] = fc_piece(c - 1, 0)
                        fts[1] = fc_piece(c - 1, 1)
                    elif trel == 1:
                        fts[2] = fc_piece(c - 1, 2)
                        zdw = zdpool.tile([128, 4 * DH], BF16, tag="zd", name="zd")
                        zd_half(fts, zdw, 0)
                    elif trel == 2:
                        zd_half(fts, zdw, 1)
                    elif trel == 3:
                        zgw = gather_dial(zdw)
                if 3 <= c < NCH + 3 and trel == 3:
                    head(c - 3)
            if c + 1 < NCH:
                zmcur = gather_mod(ztnext)
            ztnext = [None] * 3
            zgd = zgw
        endpass()

    nc.compile()
    _CACHE["nc"] = nc
    return nc


def _prep_core(inputs, core):
    """Build the per-core input map (host-side shard/transpose/pad/bf16)."""
    d = {}
    sl = slice(core * BSH, (core + 1) * BSH)
    for s in range(3):
        D = IN_DIMS[s]
        shard = np.asarray(inputs[f"mod{s}"][sl], np.float32)  # [32, T, D]
        xts = np.zeros((DPAD[s], TB), np.float32)
        xts[:D] = shard.transpose(2, 1, 0).reshape(D, TB)
        gw = GW[s]
        wt = np.zeros((DPAD[s], 4 * gw), np.float32)
        wt[:D] = _gate_reorder_T(np.asarray(inputs[f"w_ih{s}"], np.float32), HID[s], gw)
        bias = _gate_reorder_b_w(
            np.asarray(inputs[f"b_ih{s}"], np.float32)
            + np.asarray(inputs[f"b_hh{s}"], np.float32),
            HID[s],
            gw,
        )
        if s == 2:
            d["bias2t"] = np.broadcast_to(bias, (128, 4 * gw)).copy()
        else:
            xts[D] = 1.0
            wt[D] = bias
        d[f"xt{s}"] = _bf16(xts)
        d[f"wih{s}"] = _bf16(wt)
        d[f"whh{s}"] = _bf16(
            _gate_reorder_T(np.asarray(inputs[f"w_hh{s}"], np.float32), HID[s], gw)
        )
        d[f"fcw{s}"] = _bf16(np.asarray(inputs[f"fc_w{s}"], np.float32).T)
        d[f"fcb{s}"] = np.asarray(inputs[f"fc_b{s}"], np.float32).reshape(-1, 1).copy()
    wihdt = _gate_reorder_T(np.asarray(inputs["w_ih_d"], np.float32), DH, DH)  # [250, 1024]
    d["wihd0"] = _bf16(wihdt[0:100])
    d["wihd1"] = _bf16(wihdt[100:150])
    d["wihd2"] = _bf16(wihdt[150:250])
    bd = _gate_reorder_b(
        np.asarray(inputs["b_ih_d"], np.float32)
        + np.asarray(inputs["b_hh_d"], np.float32),
        DH,
        DH,
    )
    d["bdrow2"] = np.broadcast_to(bd, (128, 4 * DH)).copy()
    whhdt = _gate_reorder_T(np.asarray(inputs["w_hh_d"], np.float32), DH, DH)  # [256, 1024]
    d["whhd0"] = _bf16(whhdt[0:128])
    d["whhd1"] = _bf16(whhdt[128:256])
    fow = np.asarray(inputs["fc_out_w"], np.float32).T  # [256, 128]
    d["fcoutw0"] = _bf16(fow[0:128])
    d["fcoutw1"] = _bf16(fow[128:256])
    d["fcoutb"] = np.asarray(inputs["fc_out_b"], np.float32).reshape(-1, 1).copy()
    d["smaxwt"] = _bf16(np.asarray(inputs["smax_w"], np.float32).T)
    d["smaxbt"] = np.broadcast_to(
        np.asarray(inputs["smax_b"], np.float32), (128, NCLS)
    ).copy()
    d["idbf"] = _bf16(np.eye(128, dtype=np.float32))
    i32 = np.zeros((128, 32), np.float32)
    for k in range(4):
        i32[32 * k : 32 * (k + 1)] = np.eye(32)
    d["i32s"] = _bf16(i32)
    return d


def run(inputs, trace=False, **kw):
    nc = _build()
    in_maps = [_prep_core(inputs, i) for i in range(NCORES)]
    res = run_bass_kernel_spmd(nc, in_maps, list(range(NCORES)), trace=trace, **kw)
    full = np.concatenate(
        [np.asarray(res.results[i]["out"], np.float32) for i in range(NCORES)], axis=0
    )
    return full, res


def kernel(**inputs) -> np.ndarray:
    out, _ = run(inputs, trace=False)
    return out
